# revision 29
# baseline (speedup 1.0000x reference)
"""NTM cell kernel for Trainium2 (8 NeuronCores, batch-parallel), v2.

Strategy (per core, 8 batches):
  - prev_memory is cast to bf16 on the HOST, halving the HBM load (8.4MB).
  - Pipelined emission: per-batch m16 load (Pool SWDGE queue, sequential so
    batch 0 lands first) -> DMA-transpose halves (SP+ACT HWDGE queues) ->
    tensor-engine dot/square streams -> per-group addressing chains -> read
    contraction, all interleaved so chains overlap later batches' streams.
  - Chains: softmax denominators via gpsimd.partition_all_reduce (Pool),
    circular-shift boundaries via permu/permd matmuls positioned in the PE
    queue between stream batches, beta*rsqrt(|k|^2) folded into a per-batch
    scalar in the preamble.
  - Read contraction emits out[d, 2] per batch (stationary = m16 chunk), so
    no per-batch transposes are needed in the tail.
  - Only one ACT table set (exp/ln); sigmoid/tanh/softplus/rsqrt are
    rewritten via exp/ln so no table reloads occur.
"""

import sys

sys.path.insert(0, "/opt/trn_rl_repo")

import numpy as np

import concourse.bass as bass
import concourse.bass_isa as bass_isa
import concourse.tile as tile
from concourse import mybir

F32 = mybir.dt.float32
BF16 = mybir.dt.bfloat16
AF = mybir.ActivationFunctionType
OP = mybir.AluOpType
AX = mybir.AxisListType
RED = bass_isa.ReduceOp

B, N, D, C, IN, S = 64, 8192, 64, 256, 128, 3
NCORES = 8
BL = B // NCORES          # batches per core
P = 128                   # partitions
CH = N // P               # 64 chunks per batch (n = p*64 + c)
NPAIR = CH // 2           # 32 transposed pair-tiles per batch
EPS = 1e-8
GRP = 4                   # batches per chain group
NG = BL // GRP            # 2 groups

# whead column map (read head first, then write head, stride 70 between
# matching scalars of the two heads)
KR0, KR1 = 0, 64
BR, GR = 64, 65
SR0, SR1 = 66, 69
GAMR = 69
KW0, KW1 = 70, 134
BW, GW = 134, 135
SW0, SW1 = 136, 139
GAMW = 139
E0, E1 = 140, 204
A0, A1 = 204, 268
NHEAD = 268

# scalar table: READ head at 0..7, WRITE head at 8..15 (strided [q, q+8]
# writes from [BL, 2] (R, W) pairs), then cross terms.
Q_BRK_R, Q_G_R, Q_OMG_R, Q_SR0, Q_SR1, Q_SR2, Q_GAM_R, Q_SP7 = range(8)
Q_BRK_W, Q_G_W, Q_OMG_W, Q_SW0, Q_SW1, Q_SW2, Q_GAM_W, Q_SP15 = range(8, 16)
Q_AKR, Q_AA = 16, 17
NQ = 18
Q_USED = list(range(7)) + list(range(8, 15)) + [Q_AKR, Q_AA]

# ---------------------------------------------------------------------------
# workaround: the deployed walrus accepts only ONE sem-wait per instruction.
# After TileContext exits, hoist extra waits onto injected single-wait nops.
# ---------------------------------------------------------------------------
import concourse.tile as tile_mod


def _split_multi_waits(nc):
    for f in nc.m.functions:
        for b in f.blocks:
            insts = b.instructions
            i = 0
            while i < len(insts):
                ins = insts[i]
                si = getattr(ins, "sync_info", None)
                if si is None or len(si.on_wait) <= 1:
                    i += 1
                    continue
                waits = list(si.on_wait)
                ins.sync_info = mybir.SyncInfo(
                    on_wait=[waits[-1]], on_update=list(si.on_update)
                )
                eng = nc.engines[ins.engine]
                new_nops = []
                for w in waits[:-1]:
                    nop = eng.isa(
                        nc.isa.Opcode.NEURON_ISA_TPB_OPCODE_NOP, {}
                    ).ins
                    nop.sync_info = mybir.SyncInfo(on_wait=[w], on_update=[])
                    new_nops.append(nop)
                for nop in new_nops:
                    for bb2 in f.blocks:
                        try:
                            bb2.instructions.remove(nop)
                            break
                        except ValueError:
                            pass
                for k, nop in enumerate(new_nops):
                    insts.insert(i + k, nop)
                i += len(new_nops) + 1


_orig_exit = tile_mod.TileContext.__exit__


def _patched_exit(self, *a, **k):
    import os
    r = _orig_exit(self, *a, **k)
    if not os.environ.get("NTM_NO_WAITFIX"):
        _split_multi_waits(self.nc)
    return r


if not getattr(tile_mod.TileContext, "_waitfix_patched", False):
    tile_mod.TileContext.__exit__ = _patched_exit
    tile_mod.TileContext._waitfix_patched = True


# ---------------------------------------------------------------------------
# kernel body
# ---------------------------------------------------------------------------

def _build_module():
    nc = bass.Bass()

    mem = nc.dram_tensor("mem", [BL, N, D], BF16, kind="ExternalInput")
    memt = nc.dram_tensor("memt", [BL, 128, NPAIR, 128], BF16,
                          kind="ExternalInput")
    x_in = nc.dram_tensor("x", [BL, IN], F32, kind="ExternalInput")
    rv_in = nc.dram_tensor("rv", [BL, D], F32, kind="ExternalInput")
    prw_in = nc.dram_tensor("prw", [BL, N], F32, kind="ExternalInput")
    pww_in = nc.dram_tensor("pww", [BL, N], F32, kind="ExternalInput")
    wctrl = nc.dram_tensor("wctrl", [IN + D, C], F32, kind="ExternalInput")
    bctrl = nc.dram_tensor("bctrl", [C], F32, kind="ExternalInput")
    whead = nc.dram_tensor("whead", [C, NHEAD], F32, kind="ExternalInput")
    bhead = nc.dram_tensor("bhead", [NHEAD], F32, kind="ExternalInput")
    ident = nc.dram_tensor("ident", [128, 128], F32, kind="ExternalInput")
    onest = nc.dram_tensor("onest", [128, 128], F32, kind="ExternalInput")
    permu = nc.dram_tensor("permu", [128, 128], F32, kind="ExternalInput")
    permd = nc.dram_tensor("permd", [128, 128], F32, kind="ExternalInput")
    seldr = nc.dram_tensor("sel", [32, NQ * 128], F32, kind="ExternalInput")
    out_d = nc.dram_tensor("out", [BL, C + D], F32, kind="ExternalOutput")

    with tile.TileContext(nc) as tc:
        _emit(nc, tc, mem, memt, x_in, rv_in, prw_in, pww_in, wctrl, bctrl,
              whead, bhead, ident, onest, permu, permd, seldr, out_d)
    return nc


def _emit(nc, tc, mem, memt, x_in, rv_in, prw_in, pww_in, wctrl, bctrl, whead,
          bhead, ident, onest, permu, permd, seldr, out_d):
    from contextlib import ExitStack

    ctx = ExitStack()
    big = ctx.enter_context(tc.tile_pool(name="big", bufs=1))
    cons = ctx.enter_context(tc.tile_pool(name="cons", bufs=1))
    work = ctx.enter_context(tc.tile_pool(name="work", bufs=1))
    qallp = ctx.enter_context(tc.tile_pool(name="qallp", bufs=2))
    t2p = ctx.enter_context(tc.tile_pool(name="t2p", bufs=2))
    memp = ctx.enter_context(tc.tile_pool(name="memp", bufs=12))
    ps_stream = ctx.enter_context(tc.tile_pool(name="ps_stream", bufs=2, space="PSUM"))
    ps_misc = ctx.enter_context(tc.tile_pool(name="ps_misc", bufs=4, space="PSUM"))
    ps_rvp = ctx.enter_context(tc.tile_pool(name="ps_rvp", bufs=2, space="PSUM"))

    # ---------------- big memory tiles, loaded host-pre-transposed ----------
    # t16 tiles first (feed dot/square streams), then m16 (read phase only);
    # one SWDGE queue so early batches land first. A shared 12-buffer pool
    # lets m16[b>=4] reuse the buffer of t16[b-4] once its streams are done.
    t16s = [memp.tile([P, NPAIR, 128], BF16, tag="mb", name=f"t16_{b}")
            for b in range(BL)]
    m16s = []
    # batches 0/1 go through the HWDGE queues (hardware descriptor gen, no
    # Pool-engine boot latency) so the PE can start streaming within ~4us
    nc.sync.dma_start(out=t16s[0], in_=memt[0])
    nc.scalar.dma_start(out=t16s[1], in_=memt[1])
    for b in range(2, BL):
        nc.gpsimd.dma_start(out=t16s[b], in_=memt[b])

    def emit_m16_loads():
        # emitted after batches 0-3 are consumed so the buffer-pool WAR deps
        # (m16[b>=4] reuses t16[b-4]'s buffer) are tracked correctly
        for b in range(BL):
            m16s.append(memp.tile([P, CH, D], BF16, tag="mb",
                                  name=f"m16_{b}"))
            nc.gpsimd.dma_start(
                out=m16s[b], in_=mem[b].rearrange("(p c) d -> p c d", p=128)
            )

    # ---------------- constants / weights to SBUF (split SP/ACT queues) ----
    ident_sb = cons.tile([128, 128], F32, tag="ident")
    nc.sync.dma_start(out=ident_sb, in_=ident[:])
    wc0 = cons.tile([128, C], F32, tag="wc0")
    nc.sync.dma_start(out=wc0, in_=wctrl[0:128, :])
    wc1 = cons.tile([64, C], F32, tag="wc1")
    nc.sync.dma_start(out=wc1, in_=wctrl[128:192, :])
    bc_sb = cons.tile([128, 2], F32, tag="bc")
    nc.sync.dma_start(out=bc_sb, in_=bctrl.rearrange("(j p) -> p j", p=128))
    xt_in = cons.tile([BL, IN], F32, tag="xt_in")
    nc.sync.dma_start(out=xt_in, in_=x_in[:])
    rv_sb = cons.tile([BL, D], F32, tag="rv_sb")
    nc.sync.dma_start(out=rv_sb, in_=rv_in[:])

    wh0 = cons.tile([128, NHEAD], F32, tag="wh0")
    nc.scalar.dma_start(out=wh0, in_=whead[0:128, :])
    wh1 = cons.tile([128, NHEAD], F32, tag="wh1")
    nc.scalar.dma_start(out=wh1, in_=whead[128:256, :])
    bh_sb = cons.tile([1, NHEAD], F32, tag="bh")
    nc.scalar.dma_start(out=bh_sb, in_=bhead.rearrange("(o n) -> o n", o=1))
    ones_sb = cons.tile([128, 128], F32, tag="ones")
    nc.scalar.dma_start(out=ones_sb, in_=onest[:])
    permu_sb = cons.tile([128, 128], F32, tag="permu")
    nc.scalar.dma_start(out=permu_sb, in_=permu[:])
    permd_sb = cons.tile([128, 128], F32, tag="permd")
    nc.scalar.dma_start(out=permd_sb, in_=permd[:])
    sel_sb = cons.tile([32, NQ * 128], F32, tag="sel")
    nc.scalar.dma_start(out=sel_sb, in_=seldr[:])
    pw_w = cons.tile([128, BL, CH], F32, tag="pw_w")
    nc.scalar.dma_start(out=pw_w, in_=pww_in.rearrange("b (p c) -> p b c", p=128))
    pw_r = cons.tile([128, BL, CH], F32, tag="pw_r")
    nc.scalar.dma_start(out=pw_r, in_=prw_in.rearrange("b (p c) -> p b c", p=128))

    # ---------------- controller: hT = relu(W_ctrl^T @ ctrl_in^T + b) -------
    ps_xt = ps_misc.tile([128, 144], F32, tag="pm")
    nc.tensor.transpose(ps_xt[:, 0:BL], xt_in, ident_sb[0:BL, 0:BL])
    xT = work.tile([128, BL], F32, tag="xT")
    nc.vector.tensor_copy(xT, ps_xt[:, 0:BL])
    ps_rt = ps_misc.tile([128, 144], F32, tag="pm")
    nc.tensor.transpose(ps_rt[0:D, 0:BL], rv_sb, ident_sb[0:BL, 0:BL])
    rvT = work.tile([64, BL], F32, tag="rvT")
    nc.vector.tensor_copy(rvT, ps_rt[0:D, 0:BL])

    hT_sb = []
    for j in range(2):
        ps_h = ps_misc.tile([128, 144], F32, tag="pm")
        nc.tensor.matmul(ps_h[:, 0:BL], wc0[:, j * 128:(j + 1) * 128], xT,
                         start=True, stop=False)
        nc.tensor.matmul(ps_h[:, 0:BL], wc1[:, j * 128:(j + 1) * 128], rvT,
                         start=False, stop=True)
        h_j = work.tile([128, BL], F32, tag=f"hT{j}")
        nc.scalar.activation(h_j, ps_h[:, 0:BL], AF.Relu,
                             bias=bc_sb[:, j:j + 1], scale=1.0)
        hT_sb.append(h_j)

    # ---------------- head params P = h @ Whead + bhead ----------------
    ps_p = ps_misc.tile([BL, 512], F32, tag="pm")
    nc.tensor.matmul(ps_p[:, 0:NHEAD], hT_sb[0], wh0, start=True, stop=False)
    nc.tensor.matmul(ps_p[:, 0:NHEAD], hT_sb[1], wh1, start=False, stop=False)
    nc.tensor.matmul(ps_p[:, 0:NHEAD], ones_sb[0:1, 0:BL], bh_sb,
                     start=False, stop=True)
    p_sb = work.tile([BL, NHEAD], F32, tag="p_sb")
    nc.vector.tensor_copy(p_sb, ps_p[:, 0:NHEAD])

    # ---------------- VA: per-batch d-vectors [BL, 8*64] ----------------
    # vec order: 0 k_w, 1 k_r, 2 e*k_r, 3 a, 4 a*e, 5 ones, 6 e, 7 e^2
    va = work.tile([BL, 512], F32, tag="va")
    nc.vector.tensor_copy(va[:, 0:64], p_sb[:, KW0:KW1])
    nc.vector.tensor_copy(va[:, 64:128], p_sb[:, KR0:KR1])

    def _sigmoid(dst, src):  # dst = 1/(1+exp(-src))
        nc.scalar.activation(dst, src, AF.Exp, scale=-1.0)
        nc.vector.tensor_scalar_add(dst, dst, 1.0)
        nc.vector.reciprocal(dst, dst)

    # e = sigmoid(P_e) -> va[:, 384:448]
    _sigmoid(va[:, 384:448], p_sb[:, E0:E1])
    # a = tanh(P_a) = 1 - 2/(exp(2x)+1) -> va[:, 192:256]
    nc.scalar.activation(va[:, 192:256], p_sb[:, A0:A1], AF.Exp, scale=2.0)
    nc.vector.tensor_scalar_add(va[:, 192:256], va[:, 192:256], 1.0)
    nc.vector.reciprocal(va[:, 192:256], va[:, 192:256])
    nc.vector.tensor_scalar(va[:, 192:256], va[:, 192:256], -2.0, 1.0,
                            op0=OP.mult, op1=OP.add)
    # e*k_r, a*e, ones, e^2
    nc.vector.tensor_mul(va[:, 128:192], va[:, 384:448], va[:, 64:128])
    nc.vector.tensor_mul(va[:, 256:320], va[:, 192:256], va[:, 384:448])
    nc.vector.memset(va[:, 320:384], 1.0)
    nc.vector.tensor_mul(va[:, 448:512], va[:, 384:448], va[:, 384:448])

    # ---------------- VTD: transposed vectors with zero-halves --------------
    vtd = work.tile([128, 2, 8, BL], BF16, tag="vtd")
    nc.vector.memset(vtd, 0.0)
    vapad = work.tile([BL, 8, 128], F32, tag="vapad")
    nc.vector.memset(vapad, 0.0)
    for v in range(8):
        nc.vector.tensor_copy(vapad[:, v, 64:128], va[:, v * 64:(v + 1) * 64])
    ps_top = ps_misc.tile([128, 144], F32, tag="pm")
    ps_bot = ps_misc.tile([128, 144], F32, tag="pm")
    for v in range(8):
        nc.tensor.transpose(ps_top[0:64, v * BL:(v + 1) * BL],
                            va[:, v * 64:(v + 1) * 64],
                            ident_sb[0:BL, 0:BL])
        nc.tensor.transpose(ps_bot[:, v * BL:(v + 1) * BL],
                            vapad[:, v, :], ident_sb[0:BL, 0:BL])
    nc.vector.tensor_copy(
        vtd[0:64].rearrange("p h v b -> p (h v b)")[:, 0:64],
        ps_top[0:64, 0:64])
    nc.vector.tensor_copy(
        vtd[64:128].rearrange("p h v b -> p (h v b)")[:, 64:128],
        ps_bot[64:128, 0:64])
    # f32 copies of e^T and a^T for the read-vector assembly
    eT_sb = work.tile([64, BL], F32, tag="eT_sb")
    nc.vector.tensor_copy(eT_sb, ps_top[0:64, 6 * BL:7 * BL])
    aT_sb = work.tile([64, BL], F32, tag="aT_sb")
    nc.vector.tensor_copy(aT_sb, ps_top[0:64, 3 * BL:4 * BL])

    # ---------------- per-batch scalars S8 [BL, 32], both heads batched ----
    # p_sb head scalars interleave at stride 70: col q -> (R at q, W at q+70)
    s8 = work.tile([BL, 32], F32, tag="s8")
    nc.vector.memset(s8, 0.0)
    tmp64 = work.tile([BL, 64], F32, tag="tmp64")
    t2w = work.tile([BL, 2, 3], F32, tag="t2w")

    def hp2(col0, n=1):
        base = p_sb[:, col0:col0 + 1]
        ap = [base.ap[0], [70, 2]] + ([[1, n]] if n > 1 else [])
        return bass.AP(tensor=base.tensor, offset=base.offset, ap=ap)

    def s8w(q, n=1):
        base = s8[:, q:q + 1]
        ap = [base.ap[0], [8, 2]] + ([[1, n]] if n > 1 else [])
        return bass.AP(tensor=base.tensor, offset=base.offset, ap=ap)

    # softplus(beta) for both heads -> t2w[:, :, 0]
    nc.scalar.activation(t2w[:, :, 0], hp2(BR), AF.Exp)
    nc.vector.tensor_scalar_add(t2w[:, :, 0], t2w[:, :, 0], 1.0)
    nc.scalar.activation(t2w[:, :, 0], t2w[:, :, 0], AF.Ln)
    # |k|^2 for both heads -> t2w[:, :, 1]  (va: k_w at 0, k_r at 64)
    nc.vector.tensor_mul(tmp64, va[:, 64:128], va[:, 64:128])
    nc.vector.reduce_sum(t2w[:, 0, 1:2], tmp64, axis=AX.X)
    nc.vector.tensor_mul(tmp64, va[:, 0:64], va[:, 0:64])
    nc.vector.reduce_sum(t2w[:, 1, 1:2], tmp64, axis=AX.X)
    # rsqrt via exp(-0.5 ln), then BRK = softplus(beta) * rsqrt(|k|^2)
    nc.scalar.activation(t2w[:, :, 1], t2w[:, :, 1], AF.Ln)
    nc.scalar.activation(t2w[:, :, 1], t2w[:, :, 1], AF.Exp, scale=-0.5)
    nc.vector.tensor_mul(t2w[:, :, 2], t2w[:, :, 0], t2w[:, :, 1])
    nc.vector.tensor_copy(s8w(Q_BRK_R), t2w[:, :, 2])
    # g = sigmoid, omg = 1-g for both heads
    nc.scalar.activation(t2w[:, :, 0], hp2(GR), AF.Exp, scale=-1.0)
    nc.vector.tensor_scalar_add(t2w[:, :, 0], t2w[:, :, 0], 1.0)
    nc.vector.reciprocal(t2w[:, :, 0], t2w[:, :, 0])
    nc.vector.tensor_copy(s8w(Q_G_R), t2w[:, :, 0])
    nc.vector.tensor_scalar(t2w[:, :, 0], t2w[:, :, 0], -1.0, 1.0,
                            op0=OP.mult, op1=OP.add)
    nc.vector.tensor_copy(s8w(Q_OMG_R), t2w[:, :, 0])
    # softmax3 over shift logits for both heads
    ex3 = work.tile([BL, 2, 3], F32, tag="ex3")
    nc.scalar.activation(ex3, hp2(SR0, 3), AF.Exp)
    sm2 = work.tile([BL, 2], F32, tag="sm2")
    nc.vector.reduce_sum(sm2, ex3, axis=AX.X)
    nc.vector.reciprocal(sm2, sm2)
    sm2b = bass.AP(tensor=sm2[:, :].tensor, offset=sm2[:, :].offset,
                   ap=[sm2[:, :].ap[0], sm2[:, :].ap[1], [0, 3]])
    nc.vector.tensor_mul(t2w, ex3, sm2b)
    nc.vector.tensor_copy(s8w(Q_SR0, 3), t2w)
    # gamma = softplus + 1 for both heads
    nc.scalar.activation(t2w[:, :, 0], hp2(GAMR), AF.Exp)
    nc.vector.tensor_scalar_add(t2w[:, :, 0], t2w[:, :, 0], 1.0)
    nc.scalar.activation(t2w[:, :, 0], t2w[:, :, 0], AF.Ln)
    nc.vector.tensor_scalar_add(t2w[:, :, 0], t2w[:, :, 0], 1.0)
    nc.vector.tensor_copy(s8w(Q_GAM_R), t2w[:, :, 0])
    # cross terms: <a, k_r>, <a, a>
    nc.vector.tensor_mul(tmp64, va[:, 192:256], va[:, 64:128])
    nc.vector.reduce_sum(s8[:, Q_AKR:Q_AKR + 1], tmp64, axis=AX.X)
    nc.vector.tensor_mul(tmp64, va[:, 192:256], va[:, 192:256])
    nc.vector.reduce_sum(s8[:, Q_AA:Q_AA + 1], tmp64, axis=AX.X)

    # transpose S8 -> SC [32, BL] and broadcast -> bc_all [128, NQ*BL]
    ps_sc = ps_misc.tile([128, 144], F32, tag="pm")
    nc.tensor.transpose(ps_sc[0:32, 0:BL], s8, ident_sb[0:BL, 0:BL])
    sc_sb = work.tile([32, BL], F32, tag="sc_sb")
    nc.vector.tensor_copy(sc_sb, ps_sc[0:32, 0:BL])
    ps_bc = ps_misc.tile([128, 144], F32, tag="pm")
    for q in range(NQ):
        nc.tensor.matmul(ps_bc[:, q * BL:(q + 1) * BL],
                         sel_sb[:, q * 128:(q + 1) * 128], sc_sb,
                         start=True, stop=True)
    bc_all = work.tile([128, NQ * BL], F32, tag="bc_all")
    nc.vector.tensor_copy(bc_all, ps_bc[:, 0:NQ * BL])

    def BCC(q, gs, n):
        return bc_all[:, q * BL + gs:q * BL + gs + n]

    # ---------------- output staging for h ----------------
    out_sb = work.tile([BL, C + D], F32, tag="out_sb")
    ps_ho = ps_misc.tile([128, 144], F32, tag="pm")
    nc.tensor.transpose(ps_ho[0:BL, 0:128], hT_sb[0], ident_sb)
    nc.vector.tensor_copy(out_sb[:, 0:128], ps_ho[0:BL, 0:128])
    ps_ho2 = ps_misc.tile([128, 144], F32, tag="pm")
    nc.tensor.transpose(ps_ho2[0:BL, 0:128], hT_sb[1], ident_sb)
    nc.vector.tensor_copy(out_sb[:, 128:256], ps_ho2[0:BL, 0:128])

    rv12 = work.tile([64, BL, 2], F32, tag="rv12")
    swb = work.tile([64, BL], F32, tag="swb")

    # ---------------- helpers for the heavy phase ----------------
    def scb4(q, gs):
        base = bc_all[:, q * BL + gs:q * BL + gs + GRP]
        return bass.AP(tensor=base.tensor, offset=base.offset,
                       ap=[base.ap[0], base.ap[1], [0, 32], [0, 2]])

    def scb3(q, gs, n=CH):
        base = bc_all[:, q * BL + gs:q * BL + gs + GRP]
        return bass.AP(tensor=base.tensor, offset=base.offset,
                       ap=[base.ap[0], base.ap[1], [0, n]])

    def bc3(t8, n=CH):
        base = t8[:, :]
        return bass.AP(tensor=base.tensor, offset=base.offset,
                       ap=[base.ap[0], base.ap[1], [0, n]])

    def c4(t):
        return t.rearrange("p b (u w) -> p b u w", w=2)

    def ctile(tag):
        return work.tile([P, GRP, CH], F32, tag=tag, name=tag)

    def wtile(tag):
        return work.tile([128, GRP], F32, tag=tag, name=tag)

    def colsum_bcast(cs8, eps, tag):
        # one ones-stationary matmul sums over partitions and broadcasts
        # the per-batch total to every output partition
        ps_t = ps_misc.tile([128, 144], F32, tag="pm")
        nc.tensor.matmul(ps_t[:, 0:GRP], ones_sb, cs8, start=True, stop=True)
        rt = wtile(tag)
        if eps is not None:
            nc.vector.tensor_scalar_add(rt, ps_t[:, 0:GRP], eps)
            nc.vector.reciprocal(rt, rt)
        else:
            nc.vector.reciprocal(rt, ps_t[:, 0:GRP])
        return rt

    def w_chain_a(dk_v, ssm_v, qo, gs):
        """Chain part A: content softmax through its denominator colsum."""
        brk, g_ = qo, qo + 1
        lv = ctile("wc_lv")
        nc.scalar.activation(c4(lv), ssm_v, AF.Ln)
        inv = ctile("wc_inv")
        nc.scalar.activation(inv, lv, AF.Exp, scale=-0.5)
        bs1 = ctile("wc_bs1")
        nc.vector.tensor_mul(c4(bs1), dk_v, scb4(brk, gs))
        bsim = ctile("wc_bsim")
        nc.vector.tensor_mul(bsim, bs1, inv)
        ex = ctile("wc_ex")
        nc.scalar.activation(ex, bsim, AF.Exp)
        cs = wtile("wc_cs")
        nc.vector.reduce_sum(cs, ex, axis=AX.X)
        rtot = colsum_bcast(cs, None, "wc_rt1")  # PE op at hook position
        gt = wtile("wc_gt")
        nc.vector.tensor_mul(gt, rtot, BCC(g_, gs, GRP))
        return dict(ex=ex, gt=gt)

    def w_chain_b(st, pw_all, qo, gs):
        """Chain part B: gate+interp, shift matmul (PE at hook), ws, cs2."""
        omg, s0, s1, s2, gam = qo + 2, qo + 3, qo + 4, qo + 5, qo + 6
        ex, gt = st["ex"], st["gt"]
        t9 = ctile("wc_t9")
        nc.vector.tensor_mul(t9, pw_all, scb3(omg, gs))
        wg = ctile("wc_wg")
        nc.vector.tensor_mul(wg, ex, bc3(gt))
        nc.vector.tensor_add(wg, wg, t9)
        ps_sh = ps_misc.tile([128, 144], F32, tag="pm")
        nc.tensor.matmul(ps_sh[:, 0:GRP], permu_sb, wg[:, :, 0],
                         start=True, stop=True)
        nc.tensor.matmul(ps_sh[:, GRP:2 * GRP], permd_sb, wg[:, :, CH - 1],
                         start=True, stop=True)
        ws = ctile("wc_ws")
        nc.vector.tensor_mul(ws, wg, scb3(s1, gs))
        t10 = ctile("wc_t10")
        nc.vector.tensor_mul(t10[:, :, 0:CH - 1], wg[:, :, 1:CH],
                             scb3(s0, gs, CH - 1))
        nc.vector.tensor_add(ws[:, :, 0:CH - 1], ws[:, :, 0:CH - 1],
                             t10[:, :, 0:CH - 1])
        nc.vector.tensor_mul(t10[:, :, 1:CH], wg[:, :, 0:CH - 1],
                             scb3(s2, gs, CH - 1))
        nc.vector.tensor_add(ws[:, :, 1:CH], ws[:, :, 1:CH],
                             t10[:, :, 1:CH])
        # boundary fixups: ps_sh [0:G]=wg[p+1,:,0]*s0, [G:2G]=wg[p-1,:,CH-1]*s2
        shm = work.tile([128, 2, GRP], F32, tag="wc_shm", name="wc_shm")
        b0 = bc_all[:, s0 * BL + gs:s0 * BL + gs + GRP]
        nc.vector.tensor_mul(
            shm,
            ps_sh[:, 0:2 * GRP].rearrange("p (h g) -> p h g", h=2),
            bass.AP(tensor=b0.tensor, offset=b0.offset,
                    ap=[b0.ap[0], [(s2 - s0) * BL, 2], b0.ap[1]]))
        nc.vector.tensor_add(ws[:, :, CH - 1], ws[:, :, CH - 1], shm[:, 0, :])
        nc.vector.tensor_add(ws[:, :, 0], ws[:, :, 0], shm[:, 1, :])
        # sharpen
        lg = ctile("wc_lg")
        nc.scalar.activation(lg, ws, AF.Ln)
        nc.vector.tensor_mul(lg, lg, scb3(gam, gs))
        wp = ctile("wc_wp")
        nc.scalar.activation(wp, lg, AF.Exp)
        cs2 = wtile("wc_cs2")
        nc.vector.reduce_sum(cs2, wp, axis=AX.X)
        st["wp"], st["cs2"] = wp, cs2

    def w_chain_c(st, gs, dst):
        """Chain part C: final normalize (colsum PE op at hook position)."""
        rt2 = colsum_bcast(st["cs2"], EPS, "wc_rt2")
        nc.vector.tensor_mul(dst, st["wp"], bc3(rt2))

    def algebra(gi, qall, w_w, dots_r, ss_r):
        gs = gi * GRP
        q4 = qall.rearrange("p b (cp j) -> p b cp j", j=16)

        def QV(q):
            return q4[:, :, :, 2 * q:2 * q + 2]

        t_a = ctile("alg_t")
        nc.vector.tensor_scalar(c4(t_a), QV(2), -1.0, None, op0=OP.mult)
        nc.vector.tensor_add(t_a, t_a, scb3(Q_AKR, gs))
        nc.vector.tensor_mul(t_a, w_w, t_a)
        nc.vector.tensor_add(c4(dots_r), c4(t_a), QV(1))

        a1 = ctile("alg_a1")
        nc.vector.tensor_sub(c4(a1), QV(3), QV(6))  # sma - sme
        a2 = ctile("alg_a2")
        nc.vector.tensor_scalar(c4(a2), QV(4), -2.0, None, op0=OP.mult)
        nc.vector.tensor_add(a2, a2, scb3(Q_AA, gs))
        nc.vector.tensor_add(c4(a2), c4(a2), QV(7))  # + sme2
        h1 = ctile("alg_h1")
        nc.vector.tensor_mul(h1, w_w, a2)
        t_b = ctile("alg_tb")
        nc.vector.tensor_scalar(t_b, a1, 2.0, None, op0=OP.mult)
        nc.vector.tensor_add(h1, h1, t_b)
        nc.vector.tensor_mul(h1, w_w, h1)
        nc.vector.tensor_add(c4(ss_r), c4(h1), QV(5))  # + ssm

    def read_group(gi, w_r, w_w):
        gs = gi * GRP
        wrw = ctile(f"wrw{gi}")
        nc.vector.tensor_mul(wrw, w_r, w_w)
        swc = wtile(f"swc{gi}")
        nc.vector.reduce_sum(swc, wrw, axis=AX.X)
        ps_sw = ps_misc.tile([128, 144], F32, tag="pm")
        nc.tensor.matmul(ps_sw[:, 0:GRP], ones_sb, swc, start=True, stop=True)
        nc.scalar.copy(swb[:, gs:gs + GRP], ps_sw[0:64, 0:GRP])

        wrv = work.tile([P, GRP, CH, 2], BF16, tag=f"wrv{gi}",
                        name=f"wrv{gi}")
        nc.vector.tensor_copy(wrv[:, :, :, 0], w_r)
        nc.vector.tensor_copy(wrv[:, :, :, 1], wrw)
        for bb in range(GRP):
            b = gs + bb
            ps_rv = ps_rvp.tile([64, 2], F32, tag="ps_rv")
            for c in range(CH):
                nc.tensor.matmul(ps_rv, m16s[b][:, c, :], wrv[:, bb, c, :],
                                 start=(c == 0), stop=(c == CH - 1))
            nc.scalar.copy(rv12[:, b, :], ps_rv)

    # ---------------- stream + chain pipeline ----------------
    rhs_m = [vtd[:, :, 0:5, b].rearrange("p h v -> p v h") for b in range(BL)]
    rhs_s = [vtd[:, :, 5:8, b].rearrange("p h v -> p v h") for b in range(BL)]
    qalls = [qallp.tile([P, GRP, 512], F32, tag="qall", name=f"qall{gi}")
             for gi in range(NG)]
    wws = [work.tile([P, GRP, CH], F32, tag=f"w_w{gi}", name=f"w_w{gi}")
           for gi in range(NG)]
    wrs = [work.tile([P, GRP, CH], F32, tag=f"w_r{gi}", name=f"w_r{gi}")
           for gi in range(NG)]
    drs = [work.tile([P, GRP, CH], F32, tag=f"dr{gi}", name=f"dr{gi}")
           for gi in range(NG)]
    srs = [work.tile([P, GRP, CH], F32, tag=f"sr{gi}", name=f"sr{gi}")
           for gi in range(NG)]

    # square engines per batch: DVE early (free before chains), ACT late
    SQ_ENG = [nc.vector, nc.vector, nc.vector, nc.vector,
              nc.scalar, nc.scalar, nc.scalar, nc.scalar]

    def emit_batch(b):
        gi, bb = b // GRP, b % GRP
        t16b = t16s[b]
        pb = ps_stream.tile([128, 512], F32, tag="pb")
        for cp in range(NPAIR):
            nc.tensor.matmul(pb[:, cp * 16:cp * 16 + 10],
                             t16b[:, cp], rhs_m[b], start=True, stop=True)
        for g in range(2):
            t2 = t2p.tile([P, 16, 128], BF16, tag="t2")
            sq_src = t16b[:, g * 16:(g + 1) * 16].rearrange("p a q -> p (a q)")
            sq_dst = t2.rearrange("p a q -> p (a q)")
            eng = SQ_ENG[b]
            if eng is nc.scalar:
                nc.scalar.activation(sq_dst, sq_src, AF.Square)
            else:
                eng.tensor_mul(sq_dst, sq_src, sq_src)
            for cp in range(g * 16, (g + 1) * 16):
                nc.tensor.matmul(pb[:, cp * 16 + 10:cp * 16 + 16],
                                 t2[:, cp - g * 16], rhs_s[b],
                                 start=True, stop=True)
        nc.scalar.copy(qalls[gi][:, bb, :], pb)

    def qdk(gi):
        return qalls[gi].rearrange("p b (cp j) -> p b cp j", j=16)[:, :, :, 0:2]

    def qssm(gi):
        return qalls[gi].rearrange("p b (cp j) -> p b cp j", j=16)[:, :, :, 10:12]

    # group 0 chains interleave with group 1's streams; each chain's PE ops
    # (colsums + shift) land between stream batches in the PE queue
    emit_batch(0)
    emit_batch(1)
    emit_batch(2)
    emit_batch(3)
    emit_m16_loads()
    emit_batch(4)
    st_w0 = w_chain_a(qdk(0), qssm(0), 8, 0)
    emit_batch(5)
    w_chain_b(st_w0, pw_w[:, 0:GRP], 8, 0)
    emit_batch(6)
    w_chain_c(st_w0, 0, wws[0])
    algebra(0, qalls[0], wws[0], drs[0], srs[0])
    st_r0 = w_chain_a(c4(drs[0]), c4(srs[0]), 0, 0)
    emit_batch(7)
    w_chain_b(st_r0, pw_r[:, 0:GRP], 0, 0)
    w_chain_c(st_r0, 0, wrs[0])
    # group-1 chains interleave with group-0's read contraction
    st_w1 = w_chain_a(qdk(1), qssm(1), 8, GRP)
    read_group(0, wrs[0], wws[0])
    w_chain_b(st_w1, pw_w[:, GRP:2 * GRP], 8, GRP)
    w_chain_c(st_w1, GRP, wws[1])
    algebra(1, qalls[1], wws[1], drs[1], srs[1])
    st_r1 = w_chain_a(c4(drs[1]), c4(srs[1]), 0, GRP)
    w_chain_b(st_r1, pw_r[:, GRP:2 * GRP], 0, GRP)
    w_chain_c(st_r1, GRP, wrs[1])
    read_group(1, wrs[1], wws[1])

    # ---------------- read-vector assembly (all batches) ----------------
    rvt = work.tile([64, BL], F32, tag="rvt", name="rvt")
    nc.vector.tensor_mul(rvt, eT_sb, rv12[:, :, 1])  # e * r2
    nc.vector.tensor_sub(rvt, rv12[:, :, 0], rvt)    # r1 - e*r2
    m3 = work.tile([64, BL], F32, tag="m3", name="m3")
    nc.vector.tensor_mul(m3, aT_sb, swb)             # a * sum(wr*ww)
    nc.vector.tensor_add(rvt, rvt, m3)
    ps_rvo = ps_misc.tile([128, 144], F32, tag="pm")
    nc.tensor.transpose(ps_rvo[0:BL, 0:64], rvt, ident_sb[0:64, 0:64])
    nc.vector.tensor_copy(out_sb[:, C:C + D], ps_rvo[0:BL, 0:64])

    nc.sync.dma_start(out=out_d[:], in_=out_sb)
    ctx.close()


# ---------------------------------------------------------------------------
# host-side driver
# ---------------------------------------------------------------------------
_NC = None


def _get_module():
    global _NC
    if _NC is None:
        _NC = _build_module()
    return _NC


def _consts():
    ident = np.eye(128, dtype=np.float32)
    onest = np.ones((128, 128), np.float32)
    permu = np.zeros((128, 128), np.float32)
    permd = np.zeros((128, 128), np.float32)
    for m in range(128):
        permu[(m + 1) % 128, m] = 1.0
        permd[(m - 1) % 128, m] = 1.0
    sel = np.zeros((32, NQ * 128), np.float32)
    for q in range(NQ):
        sel[q, q * 128:(q + 1) * 128] = 1.0
    return ident, onest, permu, permd, sel


def build_in_maps(inputs):
    import ml_dtypes

    f = lambda k: np.ascontiguousarray(np.asarray(inputs[k], np.float32))
    whead = np.concatenate([
        f("Wk_r"), f("Wb_r"), f("Wg_r"), f("Ws_r"), f("Wgam_r"),
        f("Wk_w"), f("Wb_w"), f("Wg_w"), f("Ws_w"), f("Wgam_w"),
        f("We_w"), f("Wa_w")], axis=1)
    bhead = np.concatenate([
        f("bk_r"), f("bb_r"), f("bg_r"), f("bs_r"), f("bgam_r"),
        f("bk_w"), f("bb_w"), f("bg_w"), f("bs_w"), f("bgam_w"),
        f("be_w"), f("ba_w")])
    ident, onest, permu, permd, sel = _consts()

    mem16 = f("prev_memory").astype(ml_dtypes.bfloat16)
    # host-side transpose into the exact SBUF t16 pair-tile layout:
    # memt[b, par*64+d, cp, p] = mem[b, p*64 + 2*cp + par, d]
    memt = np.ascontiguousarray(
        mem16.reshape(B, 128, NPAIR, 2, D).transpose(0, 3, 4, 2, 1)
        .reshape(B, 128, NPAIR, 128))
    x = f("x")
    rv = f("prev_read_vector")
    prw = f("prev_read_weights")
    pww = f("prev_write_weights")
    shared = dict(wctrl=f("W_ctrl"), bctrl=f("b_ctrl"), whead=whead,
                  bhead=bhead, ident=ident, onest=onest, permu=permu,
                  permd=permd, sel=sel)
    in_maps = []
    for c in range(NCORES):
        sl = slice(c * BL, (c + 1) * BL)
        in_maps.append(dict(
            mem=np.ascontiguousarray(mem16[sl]),
            memt=np.ascontiguousarray(memt[sl]),
            x=np.ascontiguousarray(x[sl]),
            rv=np.ascontiguousarray(rv[sl]),
            prw=np.ascontiguousarray(prw[sl]),
            pww=np.ascontiguousarray(pww[sl]),
            **shared))
    return in_maps


def kernel(**inputs):
    from concourse.bass_utils import run_bass_kernel_spmd

    nc = _get_module()
    in_maps = build_in_maps(inputs)
    res = run_bass_kernel_spmd(nc, in_maps, list(range(NCORES)))
    return np.concatenate([res.results[c]["out"] for c in range(NCORES)],
                          axis=0).astype(np.float32)
